# revision 13
# baseline (speedup 1.0000x reference)
"""GCN encoder (2-layer) on 8 Trainium2 NeuronCores.

Strategy: both GCN aggregations run as dense DoubleRow fp8 matmuls on the
tensor engine (2x the bf16 column rate).  The count matrix C = A + I
(20480x20480, 0.16% dense) is materialized host-side in fp8 (small integer
counts -> exact) from the edge list, node-partitioned column-blocks across
the 8 cores.  Layer 1 is computed aggregate-first:

  h   = relu((A_hat @ x) @ W1 + s (x) b1)      s = A_hat @ 1
  out = A_hat @ (h @ W2) + s (x) b2

so the fp8-quantized operand of AGG1 is x itself (pre-scaled by dinv and
S1 host-side, e4m3), the replicated x@W1 is gone, and the small W1/W2
matmuls run on each core's local 2560-node block only.  z2 = dinv*(h@W2)
is quantized to e4m3 on-chip (scale S2 folded into W2), AllGathered in
fp8, and AGG2 also runs DoubleRow.  The D^-1/2 factors fold into per-row/
per-column scalings at PSUM eviction; biases fold in as rank-1 updates.

DMA discipline (the queues are descriptor-rate-bound at ~150ns per
per-partition descriptor): A streams as 2 MB blocks (8 k-tiles packed,
16 KB contiguous per partition per DMA), xq as 512 KB groups; the
sync queue carries only dependency-free input streams (A/xq/consts),
compute-dependent writes (z2, out) go on the scalar queue, and the
collectives plus gathered-z2 reads go on the gpsimd queue.  nt=4's A
blocks (plus one nt=3 block) stay SBUF-resident from AGG1 and are
reused by AGG2 (saves 12.6 MB of re-streaming).  A 64-byte dummy
collective fires at kernel start to absorb inter-core launch skew.
"""

import sys

sys.path.insert(0, "/opt/trn_rl_repo")

import numpy as np

N_REAL = 20000
NCORES = 8
RBLK = 2500          # real nodes per core
BLK = 2560           # padded nodes per core (20 * 128)
NPAD = NCORES * BLK  # 20480
CIN = 256
CHID = 256
COUT = 128
P = 128
KT = NPAD // 512     # 40 k-tiles over nodes
QQ = KT // 8         # 5 8-k-tile A stream blocks
NT = BLK // 512      # 5 n-tiles over a core's node block
S1 = 32.0            # fp8 scale for x (folded out via W1' = W1/S1)
S2 = 32.0            # fp8 scale for z2 (folded in via W2' = W2*S2,
                     # folded out host-side: out /= S2, b2' = b2*S2)

_compiled = None


def _build_nc():
    import concourse.bass as bass  # noqa: F401
    import concourse.mybir as mybir
    import concourse.tile as tile
    from concourse import bacc
    from contextlib import ExitStack

    f16 = mybir.dt.float16
    f8 = mybir.dt.float8e4
    f32 = mybir.dt.float32
    Alu = mybir.AluOpType
    DR = mybir.MatmulPerfMode.DoubleRow

    nc = bacc.Bacc("TRN2", target_bir_lowering=False, debug=False,
                   num_devices=NCORES)

    # External I/O, pre-tiled so every big DMA is one contiguous block
    # with >= 4 KB per partition line.
    xqT = nc.dram_tensor("xqT", [KT // 4, P, 16, CIN], f8,
                         kind="ExternalInput")
    W1 = nc.dram_tensor("W1", [P, CIN // P, CHID], f16, kind="ExternalInput")
    W2 = nc.dram_tensor("W2", [P, CHID // P, COUT], f16, kind="ExternalInput")
    Ab = nc.dram_tensor("Ab", [QQ, NT, P, 32, 512], f8,
                        kind="ExternalInput")
    sbc = nc.dram_tensor("sbc", [P, BLK], f16, kind="ExternalInput")
    dbc = nc.dram_tensor("dbc", [P, BLK], f16, kind="ExternalInput")
    dz2 = nc.dram_tensor("dz2", [P, BLK // P], f32, kind="ExternalInput")
    b1c = nc.dram_tensor("b1c", [P, CHID // P], f32, kind="ExternalInput")
    b2c = nc.dram_tensor("b2c", [P, COUT // P], f32, kind="ExternalInput")
    outT = nc.dram_tensor("outT", [P, 1, BLK], f16, kind="ExternalOutput")

    # Internal DRAM (collective buffers).  Three gather groups (t=0..1,
    # t=2..3, t=4) so each gather's dependencies are only its own z2
    # tiles and all run during AGG1's tail.  Per-core blocks are
    # partition-major so the post-gather read of a core's whole group
    # is one chunky DMA.
    bar_l = nc.dram_tensor("bar_l", [1, 16], f32)
    bar_g = nc.dram_tensor("bar_g", [NCORES, 1, 16], f32,
                           addr_space="Shared")
    z2bA = nc.dram_tensor("z2bA", [P, 2, 4, COUT], f8)
    z2bB = nc.dram_tensor("z2bB", [P, 2, 4, COUT], f8)
    z2bC = nc.dram_tensor("z2bC", [P, 1, 4, COUT], f8)
    z2gA = nc.dram_tensor("z2gA", [NCORES, P, 2, 4, COUT], f8,
                          addr_space="Shared")
    z2gB = nc.dram_tensor("z2gB", [NCORES, P, 2, 4, COUT], f8,
                          addr_space="Shared")
    z2gC = nc.dram_tensor("z2gC", [NCORES, P, 1, 4, COUT], f8,
                          addr_space="Shared")

    # Aggregation k-step q -> (core g, z2-tile t) interleaved t-major so
    # AGG2 can start on gather t=0 while later gathers are in flight.
    # Ab's tile axis is host-permuted to this order for both aggregations.
    def q_to_phys(q):
        t, g = divmod(q, NCORES)
        return g * NT + t  # physical global k-tile index

    with tile.TileContext(nc) as tc:
        with ExitStack() as octx:
            const = octx.enter_context(tc.tile_pool(name="const", bufs=1))
            s_sb = const.tile([P, BLK], f16)
            d_sb = const.tile([P, BLK], f16)
            dz2_sb = const.tile([P, BLK // P], f32)
            b1_sb = const.tile([P, CHID // P], f32)
            b2_sb = const.tile([P, COUT // P], f32)
            w1_sb = const.tile([P, CIN // P, CHID], f16)
            w2_sb = const.tile([P, CHID // P, COUT], f16)

            # consts are first read ~35us in; their DMAs are emitted after
            # the first A block so xq[0]/A[0] lead the queues at launch.
            def load_late_consts():
                nc.sync.dma_start(w1_sb[:], W1[:])
                nc.sync.dma_start(w2_sb[:], W2[:])
                nc.sync.dma_start(d_sb[:], dbc[:])
                nc.sync.dma_start(s_sb[:], sbc[:])
                nc.sync.dma_start(dz2_sb[:], dz2[:])
                nc.sync.dma_start(b1_sb[:], b1c[:])
                nc.sync.dma_start(b2_sb[:], b2c[:])

            # A-block stream pool for AGG1; bufs=6 keeps the 5 nt=4 blocks
            # plus nt=3's last block (the last allocated) valid after AGG1
            # so AGG2 reuses them from SBUF instead of re-streaming
            # 12.6 MB.  Lives for the whole kernel.
            a1_kxn = octx.enter_context(tc.tile_pool(name="a1_kxn",
                                                     bufs=6))
            z2pre_pool = octx.enter_context(
                tc.tile_pool(name="z2pre", bufs=NCORES))
            # xq pool closes after AGG1 so its 5.2 MB is reusable.
            xq_cm = tc.tile_pool(name="xqsb", bufs=KT // 4)
            xq_pool = xq_cm.__enter__()
            xq_tiles = {}   # mt//4 -> group tile [P, 16, CIN]
            zga = {}        # g -> [P, 2, 4, COUT] tile (t=0,1)
            zgb = {}        # g -> [P, 2, 4, COUT] tile (t=2,3)
            zgc = {}        # g -> [P, 1, 4, COUT] tile (t=4)
            pins = {}       # (qq, nt) -> A block handle, reused by AGG2

            # ---- Phases 1-3 fused: AGG1 + MMW1 + MMW2 + gathers ----------
            with ExitStack() as ctx:
                a1_ps = ctx.enter_context(
                    tc.tile_pool(name="a1_ps", bufs=2, space="PSUM"))
                agx_pool = ctx.enter_context(tc.tile_pool(name="agx",
                                                          bufs=2))
                h_pool = ctx.enter_context(tc.tile_pool(name="hsb",
                                                        bufs=2))
                mmh_ps = ctx.enter_context(
                    tc.tile_pool(name="mmh_ps", bufs=1, space="PSUM"))
                mmz_ps = ctx.enter_context(
                    tc.tile_pool(name="mmz_ps", bufs=1, space="PSUM"))
                a1_red = ctx.enter_context(tc.tile_pool(name="a1_red",
                                                        bufs=2))
                z2q_pool = ctx.enter_context(tc.tile_pool(name="z2q",
                                                          bufs=2))

                for nt in range(NT):
                    n0 = nt * 512
                    psums = [a1_ps.tile([P, 512], f32, name=f"a1ps{m}")
                             for m in range(2)]
                    for qq in range(QQ):
                        at = a1_kxn.tile([P, 32, 512], f8, tag="a1A")
                        if nt == NT - 1 or (nt == NT - 2 and qq == QQ - 1):
                            pins[(qq, nt)] = at
                        if nt == 0:
                            # first block lands in 4 sub-DMAs so the PE can
                            # start on the first k-tiles early; xq groups
                            # load in touch order, leading the A stream
                            for q in range(8 * qq, 8 * qq + 8):
                                g4 = q_to_phys(q) // 4
                                if g4 not in xq_tiles:
                                    xg = xq_pool.tile([P, 16, CIN], f8,
                                                      tag="xq")
                                    nc.sync.dma_start(xg[:], xqT[g4])
                                    xq_tiles[g4] = xg
                            if qq == 0:
                                for sub in range(4):
                                    nc.sync.dma_start(
                                        at[:, 8 * sub:8 * sub + 8, :],
                                        Ab[qq, nt, :,
                                           8 * sub:8 * sub + 8, :])
                            else:
                                nc.sync.dma_start(at[:], Ab[qq, nt])
                        else:
                            nc.sync.dma_start(at[:], Ab[qq, nt])
                        for oct_ in range(8):
                            q = 8 * qq + oct_
                            mt = q_to_phys(q)
                            xg = xq_tiles[mt // 4]
                            mo = (mt % 4) * 4
                            for jp in range(2):
                                for m in range(2):
                                    nc.tensor.matmul(
                                        psums[m][:],
                                        xg[:, mo + 2 * jp:mo + 2 * jp + 2,
                                           m * P:(m + 1) * P],
                                        at[:, oct_ * 4 + 2 * jp:
                                           oct_ * 4 + 2 * jp + 2, :],
                                        start=(q == 0 and jp == 0),
                                        stop=(q == KT - 1 and jp == 1),
                                        perf_mode=DR)
                        if nt == 0 and qq == 0:
                            load_late_consts()
                            nc.gpsimd.collective_compute(
                                "AllGather", mybir.AluOpType.bypass,
                                ins=[bar_l[:]], outs=[bar_g[:]],
                                replica_groups=[list(range(NCORES))])

                    # evict: aggxT = d * psum (fp16, ch-major), S1 folded
                    # into W1' host-side
                    agx = agx_pool.tile([P, 2, 512], f16, tag="agx")
                    for m in range(2):
                        nc.vector.tensor_mul(agx[:, m, :], psums[m][:],
                                             d_sb[:, n0:n0 + 512])

                    # MMW1: h = relu(aggxT.T @ W1' + b1 (x) s), ch-major
                    ht = h_pool.tile([P, 2, 512], f16, tag="h")
                    for mo in range(2):
                        psh = mmh_ps.tile([P, 512], f32, name=f"mmh{mo}")
                        for kk in range(2):
                            nc.tensor.matmul(
                                psh[:],
                                w1_sb[:, kk, mo * P:(mo + 1) * P],
                                agx[:, kk, :],
                                start=(kk == 0), stop=(kk == 1))
                        tmp = a1_red.tile([P, 512], f32, tag="a1t")
                        nc.vector.scalar_tensor_tensor(
                            tmp[:], s_sb[:, n0:n0 + 512],
                            b1_sb[:, mo:mo + 1], psh[:],
                            op0=Alu.mult, op1=Alu.add)
                        nc.vector.tensor_scalar_max(ht[:, mo, :], tmp[:],
                                                    0.0)

                    # MMW2: z2q = e4m3(d * (h.T @ W2')), node-major fp8
                    zq = z2q_pool.tile([P, 4, COUT], f8, tag="z2q")
                    ps3 = mmz_ps.tile([P, 4 * COUT], f32, name="mmz")
                    for ns in range(4):
                        psl = ps3[:, ns * COUT:(ns + 1) * COUT]
                        for mo in range(2):
                            nc.tensor.matmul(
                                psl, ht[:, mo, ns * P:(ns + 1) * P],
                                w2_sb[:, mo],
                                start=(mo == 0), stop=(mo == 1))
                        nc.vector.tensor_scalar_mul(
                            zq[:, ns, :], psl,
                            dz2_sb[:, nt * 4 + ns:nt * 4 + ns + 1])
                    # scalar-queue write: keeps this compute-dependent DMA
                    # from head-blocking the A stream on the sync queue
                    if nt < 2:
                        nc.scalar.dma_start(z2bA[:, nt], zq[:])
                    elif nt < 4:
                        nc.scalar.dma_start(z2bB[:, nt - 2], zq[:])
                    else:
                        nc.scalar.dma_start(z2bC[:, 0], zq[:])

                    if nt == 1:
                        nc.gpsimd.collective_compute(
                            "AllGather", mybir.AluOpType.bypass,
                            ins=[z2bA[:]], outs=[z2gA[:]],
                            replica_groups=[list(range(NCORES))])
                        # preload AGG2's z2 tiles as soon as each gather
                        # lands (gpsimd queue orders them right behind it)
                        for g in range(NCORES):
                            zp = z2pre_pool.tile([P, 2, 4, COUT], f8,
                                                 tag="z2preA")
                            nc.gpsimd.dma_start(zp[:], z2gA[g])
                            zga[g] = zp
                    elif nt == 3:
                        nc.gpsimd.collective_compute(
                            "AllGather", mybir.AluOpType.bypass,
                            ins=[z2bB[:]], outs=[z2gB[:]],
                            replica_groups=[list(range(NCORES))])
                        for g in range(NCORES):
                            zp = z2pre_pool.tile([P, 2, 4, COUT], f8,
                                                 tag="z2preB")
                            nc.gpsimd.dma_start(zp[:], z2gB[g])
                            zgb[g] = zp
                    elif nt == NT - 1:
                        nc.gpsimd.collective_compute(
                            "AllGather", mybir.AluOpType.bypass,
                            ins=[z2bC[:]], outs=[z2gC[:]],
                            replica_groups=[list(range(NCORES))])
                        for g in range(NCORES):
                            zp = z2pre_pool.tile([P, 1, 4, COUT], f8,
                                                 tag="z2preC")
                            nc.gpsimd.dma_start(zp[:], z2gC[g])
                            zgc[g] = zp
            xq_cm.__exit__(None, None, None)

            # ---- Phase 4: outT = d*contract(z2q, C) + b2' (x) s ----------
            # Hand-rolled k-outer loop: one PSUM bank per n-tile, so the
            # first gathered z2 tile starts compute while later gathers
            # are still in flight.  Pinned A blocks come from SBUF.
            with ExitStack() as ctx:
                a2_kxn = ctx.enter_context(
                    tc.tile_pool(name="a2_kxn", bufs=4))
                a2_red = ctx.enter_context(tc.tile_pool(name="a2_red",
                                                        bufs=2))
                a2_ps = ctx.enter_context(
                    tc.tile_pool(name="a2_ps", bufs=1, space="PSUM"))

                psums = [a2_ps.tile([P, 512], f32, name=f"a2ps{n}")
                         for n in range(NT)]
                for qq in range(QQ):
                    ats = []
                    for n in range(NT):
                        if (qq, n) in pins:
                            ats.append(pins[(qq, n)])
                            continue
                        at = a2_kxn.tile([P, 32, 512], f8, tag="a2A")
                        nc.sync.dma_start(at[:], Ab[qq, n])
                        ats.append(at)
                    for oct_ in range(8):
                        q = 8 * qq + oct_
                        t, g = divmod(q, NCORES)
                        if t < 2:
                            zt = zga[g][:, t]
                        elif t < 4:
                            zt = zgb[g][:, t - 2]
                        else:
                            zt = zgc[g][:, 0]
                        for jp in range(2):
                            for n in range(NT):
                                nc.tensor.matmul(
                                    psums[n][:],
                                    zt[:, 2 * jp:2 * jp + 2, :],
                                    ats[n][:, oct_ * 4 + 2 * jp:
                                           oct_ * 4 + 2 * jp + 2, :],
                                    start=(q == 0 and jp == 0),
                                    stop=(q == KT - 1 and jp == 1),
                                    perf_mode=DR)

                for n in range(NT):
                    n0 = n * 512
                    tmp = a2_red.tile([P, 512], f32, tag="a2t")
                    osb = a2_red.tile([P, 512], f16, tag="a2o")
                    nc.vector.tensor_mul(tmp[:], psums[n][:],
                                         d_sb[:, n0:n0 + 512])
                    nc.vector.scalar_tensor_tensor(
                        osb[:], s_sb[:, n0:n0 + 512],
                        b2_sb[:, 0:1], tmp[:],
                        op0=Alu.mult, op1=Alu.add)
                    nc.scalar.dma_start(outT[:, 0, n0:n0 + 512], osb[:])

    nc.compile()
    return nc


def _preprocess(x, edge_index, W1, b1, W2, b2):
    import ml_dtypes

    x = np.asarray(x, dtype=np.float32)
    edge_index = np.asarray(edge_index)
    W1 = np.asarray(W1, dtype=np.float32)
    b1 = np.asarray(b1, dtype=np.float32)
    W2 = np.asarray(W2, dtype=np.float32)
    b2 = np.asarray(b2, dtype=np.float32)

    row = edge_index[0].astype(np.int64)
    col = edge_index[1].astype(np.int64)

    deg = np.bincount(col, minlength=N_REAL).astype(np.float32) + 1.0
    dinv = 1.0 / np.sqrt(deg)

    idx = np.arange(N_REAL, dtype=np.int64)
    pad_id = (idx // RBLK) * BLK + idx % RBLK  # real -> padded node id

    # Dense count matrix, transposed: CT[src, dst] = A[dst, src] + I
    CT = np.zeros((NPAD, NPAD), dtype=np.uint8)
    np.add.at(CT, (pad_id[row], pad_id[col]), 1)
    CT[pad_id, pad_id] += 1
    assert CT.max() <= 16, "count exceeds exact fp8e4m3 integer range"

    # s[c] = sum_r A_hat[c, r]; dinv at padded positions -> 0
    s_real = dinv * (np.bincount(col, weights=dinv[row],
                                 minlength=N_REAL).astype(np.float32) + dinv)
    s_pad = np.zeros(NPAD, dtype=np.float32)
    s_pad[pad_id] = s_real
    dinv_pad = np.zeros(NPAD, dtype=np.float32)
    dinv_pad[pad_id] = dinv

    # xq = e4m3(S1 * dinv * x), grouped 4 k-tiles per DMA block:
    # [mt//4][p][(mt%4)*4 + j][c] = xq[mt*512 + j*128 + p, c]
    x_pad = np.zeros((NPAD, CIN), dtype=np.float32)
    x_pad[pad_id] = x
    xq_full = np.clip(S1 * dinv_pad[:, None] * x_pad, -240.0, 240.0)
    xqT_t = np.ascontiguousarray(
        xq_full.reshape(KT // 4, 4, 4, P, CIN).transpose(0, 3, 1, 2, 4)
        .reshape(KT // 4, P, 16, CIN)
    ).astype(ml_dtypes.float8_e4m3)

    W1_t = np.ascontiguousarray(
        (W1 / S1).astype(np.float16)
        .reshape(CIN // P, P, CHID).transpose(1, 0, 2))
    W2_t = np.ascontiguousarray(
        (W2 * S2).astype(np.float16)
        .reshape(CHID // P, P, COUT).transpose(1, 0, 2))
    b1_t = np.ascontiguousarray(b1.reshape(CHID // P, P).T)
    b2_t = np.ascontiguousarray((b2 * S2).reshape(COUT // P, P).T)

    in_maps = []
    for g in range(NCORES):
        C_g = CT[:, g * BLK:(g + 1) * BLK]
        # [kt][nt][p][s][n] = C_g[kt*512 + s*128 + p, nt*512 + n],
        # then permute the kt axis into the device's q-order
        # (q -> physical kt = (q % NCORES) * NT + q // NCORES) and pack
        # 8 q's per stream block: [qq][nt][p][32][512] (16 KB/partition).
        perm = [(q % NCORES) * NT + q // NCORES for q in range(KT)]
        A_t = np.ascontiguousarray(
            C_g.reshape(KT, 4, P, NT, 512).transpose(0, 3, 2, 1, 4)[perm]
            .reshape(QQ, 8, NT, P, 4, 512).transpose(0, 2, 3, 1, 4, 5)
            .reshape(QQ, NT, P, 32, 512)
        ).astype(ml_dtypes.float8_e4m3)
        s_loc = s_pad[g * BLK:(g + 1) * BLK]
        d_loc = dinv_pad[g * BLK:(g + 1) * BLK]
        s_b = np.ascontiguousarray(
            np.broadcast_to(s_loc, (P, BLK))).astype(np.float16)
        d_b = np.ascontiguousarray(
            np.broadcast_to(d_loc, (P, BLK))).astype(np.float16)
        dz2_t = np.ascontiguousarray(d_loc.reshape(BLK // P, P).T)
        in_maps.append(dict(xqT=xqT_t, W1=W1_t, W2=W2_t, Ab=A_t,
                            sbc=s_b, dbc=d_b, dz2=dz2_t,
                            b1c=b1_t, b2c=b2_t))
    return in_maps


def _run(inputs, trace=False):
    global _compiled
    if _compiled is None:
        _compiled = _build_nc()
    nc = _compiled
    from concourse.bass_utils import run_bass_kernel_spmd

    in_maps = _preprocess(**inputs)
    res = run_bass_kernel_spmd(nc, in_maps, list(range(NCORES)), trace=trace)
    out = np.empty((N_REAL, COUT), dtype=np.float32)
    for g in range(NCORES):
        out[g * RBLK:(g + 1) * RBLK] = \
            res.results[g]["outT"][:, 0, :RBLK].T.astype(np.float32) / S2
    return out, res


def kernel(**inputs) -> np.ndarray:
    out, _ = _run(inputs, trace=False)
    return out


# revision 14
# speedup vs baseline: 1.0463x; 1.0463x over previous
"""GCN encoder (2-layer) on 8 Trainium2 NeuronCores.

Strategy: both GCN aggregations run as dense DoubleRow fp8 matmuls on the
tensor engine (2x the bf16 column rate).  The count matrix C = A + I
(20480x20480, 0.16% dense) is materialized host-side in fp8 (small integer
counts -> exact) from the edge list, node-partitioned column-blocks across
the 8 cores.  Layer 1 is computed aggregate-first:

  h   = relu((A_hat @ x) @ W1 + s (x) b1)      s = A_hat @ 1
  out = A_hat @ (h @ W2) + s (x) b2

so the fp8-quantized operand of AGG1 is x itself (pre-scaled by dinv and
S1 host-side, e4m3), the replicated x@W1 is gone, and the small W1/W2
matmuls run on each core's local 2560-node block only.  z2 = dinv*(h@W2)
is quantized to e4m3 on-chip (scale S2 folded into W2), AllGathered in
fp8, and AGG2 also runs DoubleRow.  The D^-1/2 factors fold into per-row/
per-column scalings at PSUM eviction; biases fold in as rank-1 updates.

DMA discipline (the queues are descriptor-rate-bound at ~150ns per
per-partition descriptor): A streams as 2 MB blocks (8 k-tiles packed,
16 KB contiguous per partition per DMA), xq as 512 KB groups; the
sync queue carries only dependency-free input streams (A/xq/consts),
compute-dependent writes (z2, out) go on the scalar queue, and the
collectives plus gathered-z2 reads go on the gpsimd queue.  nt=4's A
blocks (plus one nt=3 block) stay SBUF-resident from AGG1 and are
reused by AGG2 (saves 12.6 MB of re-streaming).  A 64-byte dummy
collective fires at kernel start to absorb inter-core launch skew.
"""

import sys

sys.path.insert(0, "/opt/trn_rl_repo")

import numpy as np

N_REAL = 20000
NCORES = 8
RBLK = 2500          # real nodes per core
BLK = 2560           # padded nodes per core (20 * 128)
NPAD = NCORES * BLK  # 20480
CIN = 256
CHID = 256
COUT = 128
P = 128
KT = NPAD // 512     # 40 k-tiles over nodes
QQ = KT // 8         # 5 8-k-tile A stream blocks
NT = BLK // 512      # 5 n-tiles over a core's node block
S1 = 32.0            # fp8 scale for x (folded out via W1' = W1/S1)
S2 = 32.0            # fp8 scale for z2 (folded in via W2' = W2*S2,
                     # folded out host-side: out /= S2, b2' = b2*S2)

_compiled = None


def _build_nc():
    import concourse.bass as bass  # noqa: F401
    import concourse.mybir as mybir
    import concourse.tile as tile
    from concourse import bacc
    from contextlib import ExitStack

    f16 = mybir.dt.float16
    f8 = mybir.dt.float8e4
    f32 = mybir.dt.float32
    Alu = mybir.AluOpType
    DR = mybir.MatmulPerfMode.DoubleRow

    nc = bacc.Bacc("TRN2", target_bir_lowering=False, debug=False,
                   num_devices=NCORES)

    # External I/O, pre-tiled so every big DMA is one contiguous block
    # with >= 4 KB per partition line.
    xqT = nc.dram_tensor("xqT", [KT // 4, P, 16, CIN], f8,
                         kind="ExternalInput")
    W1 = nc.dram_tensor("W1", [P, CIN // P, CHID], f16, kind="ExternalInput")
    W2 = nc.dram_tensor("W2", [P, CHID // P, COUT], f16, kind="ExternalInput")
    Ab = nc.dram_tensor("Ab", [QQ, NT, P, 32, 512], f8,
                        kind="ExternalInput")
    sbc = nc.dram_tensor("sbc", [P, BLK], f16, kind="ExternalInput")
    dbc = nc.dram_tensor("dbc", [P, BLK], f16, kind="ExternalInput")
    dz2 = nc.dram_tensor("dz2", [P, BLK // P], f32, kind="ExternalInput")
    b1c = nc.dram_tensor("b1c", [P, CHID // P], f32, kind="ExternalInput")
    b2c = nc.dram_tensor("b2c", [P, COUT // P], f32, kind="ExternalInput")
    outT = nc.dram_tensor("outT", [P, 1, BLK], f16, kind="ExternalOutput")

    # Internal DRAM (collective buffers).  Three gather groups (t=0..1,
    # t=2..3, t=4) so each gather's dependencies are only its own z2
    # tiles and all run during AGG1's tail.  Per-core blocks are
    # partition-major so the post-gather read of a core's whole group
    # is one chunky DMA.
    bar_l = nc.dram_tensor("bar_l", [1, 16], f32)
    bar_g = nc.dram_tensor("bar_g", [NCORES, 1, 16], f32,
                           addr_space="Shared")
    z2bA = nc.dram_tensor("z2bA", [P, 2, 4, COUT], f8)
    z2bB = nc.dram_tensor("z2bB", [P, 2, 4, COUT], f8)
    z2bC = nc.dram_tensor("z2bC", [P, 1, 4, COUT], f8)
    z2gA = nc.dram_tensor("z2gA", [NCORES, P, 2, 4, COUT], f8,
                          addr_space="Shared")
    z2gB = nc.dram_tensor("z2gB", [NCORES, P, 2, 4, COUT], f8,
                          addr_space="Shared")
    z2gC = nc.dram_tensor("z2gC", [NCORES, P, 1, 4, COUT], f8,
                          addr_space="Shared")

    # Aggregation k-step q -> (core g, z2-tile t) interleaved t-major so
    # AGG2 can start on gather t=0 while later gathers are in flight.
    # Ab's tile axis is host-permuted to this order for both aggregations.
    def q_to_phys(q):
        t, g = divmod(q, NCORES)
        return g * NT + t  # physical global k-tile index

    with tile.TileContext(nc) as tc:
        with ExitStack() as octx:
            const = octx.enter_context(tc.tile_pool(name="const", bufs=1))
            s_sb = const.tile([P, BLK], f16)
            d_sb = const.tile([P, BLK], f16)
            dz2_sb = const.tile([P, BLK // P], f32)
            b1_sb = const.tile([P, CHID // P], f32)
            b2_sb = const.tile([P, COUT // P], f32)
            w1_sb = const.tile([P, CIN // P, CHID], f16)
            w2_sb = const.tile([P, CHID // P, COUT], f16)

            # consts are first read ~35us in; their DMAs are emitted after
            # the first A block so xq[0]/A[0] lead the queues at launch.
            def load_late_consts():
                nc.sync.dma_start(w1_sb[:], W1[:])
                nc.sync.dma_start(w2_sb[:], W2[:])
                nc.sync.dma_start(d_sb[:], dbc[:])
                nc.sync.dma_start(s_sb[:], sbc[:])
                nc.sync.dma_start(dz2_sb[:], dz2[:])
                nc.sync.dma_start(b1_sb[:], b1c[:])
                nc.sync.dma_start(b2_sb[:], b2c[:])

            # A-block stream pool for AGG1; bufs=5 keeps the 5 nt=4 blocks
            # (the last allocated) valid after AGG1 so AGG2 reuses them
            # from SBUF instead of re-streaming 10.5 MB.  The AGG2 stream
            # pool opens HERE (before the xq pool) so its address range is
            # disjoint from xq and its first blocks can stream during
            # AGG1's tail, while the PE runs on the pinned blocks and the
            # DMA queues would otherwise idle.
            a1_kxn = octx.enter_context(tc.tile_pool(name="a1_kxn",
                                                     bufs=5))
            a2_kxn = octx.enter_context(tc.tile_pool(name="a2_kxn",
                                                     bufs=2))
            z2pre_pool = octx.enter_context(
                tc.tile_pool(name="z2pre", bufs=NCORES))
            # xq pool closes after AGG1 so its 5.2 MB is reusable.
            xq_cm = tc.tile_pool(name="xqsb", bufs=KT // 4)
            xq_pool = xq_cm.__enter__()
            xq_tiles = {}   # mt//4 -> group tile [P, 16, CIN]
            zga = {}        # g -> [P, 2, 4, COUT] tile (t=0,1)
            zgb = {}        # g -> [P, 2, 4, COUT] tile (t=2,3)
            zgc = {}        # g -> [P, 1, 4, COUT] tile (t=4)
            pins = {}       # (qq, nt) -> A block handle, reused by AGG2

            # ---- Phases 1-3 fused: AGG1 + MMW1 + MMW2 + gathers ----------
            with ExitStack() as ctx:
                a1_ps = ctx.enter_context(
                    tc.tile_pool(name="a1_ps", bufs=2, space="PSUM"))
                agx_pool = ctx.enter_context(tc.tile_pool(name="agx",
                                                          bufs=2))
                h_pool = ctx.enter_context(tc.tile_pool(name="hsb",
                                                        bufs=2))
                mmh_ps = ctx.enter_context(
                    tc.tile_pool(name="mmh_ps", bufs=1, space="PSUM"))
                mmz_ps = ctx.enter_context(
                    tc.tile_pool(name="mmz_ps", bufs=1, space="PSUM"))
                a1_red = ctx.enter_context(tc.tile_pool(name="a1_red",
                                                        bufs=2))
                z2q_pool = ctx.enter_context(tc.tile_pool(name="z2q",
                                                          bufs=2))

                for nt in range(NT):
                    n0 = nt * 512
                    psums = [a1_ps.tile([P, 512], f32, name=f"a1ps{m}")
                             for m in range(2)]
                    for qq in range(QQ):
                        at = a1_kxn.tile([P, 32, 512], f8, tag="a1A")
                        if nt == NT - 1:
                            pins[(qq, nt)] = at
                        def load_xq(g4):
                            if g4 not in xq_tiles:
                                xg = xq_pool.tile([P, 16, CIN], f8,
                                                  tag="xq")
                                nc.sync.dma_start(xg[:], xqT[g4])
                                xq_tiles[g4] = xg
                        if nt == 0 and qq == 0:
                            # startup: first block lands in 4 sub-DMAs
                            # interleaved with the xq groups they need, so
                            # the PE starts on the first k-tiles early
                            for sub in range(4):
                                load_xq(q_to_phys(2 * sub) // 4)
                                load_xq(q_to_phys(2 * sub + 1) // 4)
                                nc.sync.dma_start(
                                    at[:, 8 * sub:8 * sub + 8, :],
                                    Ab[qq, nt, :, 8 * sub:8 * sub + 8, :])
                        else:
                            if nt == 0:
                                for q in range(8 * qq, 8 * qq + 8):
                                    load_xq(q_to_phys(q) // 4)
                            nc.sync.dma_start(at[:], Ab[qq, nt])
                        for oct_ in range(8):
                            q = 8 * qq + oct_
                            mt = q_to_phys(q)
                            xg = xq_tiles[mt // 4]
                            mo = (mt % 4) * 4
                            for jp in range(2):
                                for m in range(2):
                                    nc.tensor.matmul(
                                        psums[m][:],
                                        xg[:, mo + 2 * jp:mo + 2 * jp + 2,
                                           m * P:(m + 1) * P],
                                        at[:, oct_ * 4 + 2 * jp:
                                           oct_ * 4 + 2 * jp + 2, :],
                                        start=(q == 0 and jp == 0),
                                        stop=(q == KT - 1 and jp == 1),
                                        perf_mode=DR)
                        if nt == 0 and qq == 0:
                            load_late_consts()
                            nc.gpsimd.collective_compute(
                                "AllGather", mybir.AluOpType.bypass,
                                ins=[bar_l[:]], outs=[bar_g[:]],
                                replica_groups=[list(range(NCORES))])

                    # evict: aggxT = d * psum (fp16, ch-major), S1 folded
                    # into W1' host-side
                    agx = agx_pool.tile([P, 2, 512], f16, tag="agx")
                    for m in range(2):
                        nc.vector.tensor_mul(agx[:, m, :], psums[m][:],
                                             d_sb[:, n0:n0 + 512])

                    # MMW1: h = relu(aggxT.T @ W1' + b1 (x) s), ch-major
                    ht = h_pool.tile([P, 2, 512], f16, tag="h")
                    for mo in range(2):
                        psh = mmh_ps.tile([P, 512], f32, name=f"mmh{mo}")
                        for kk in range(2):
                            nc.tensor.matmul(
                                psh[:],
                                w1_sb[:, kk, mo * P:(mo + 1) * P],
                                agx[:, kk, :],
                                start=(kk == 0), stop=(kk == 1))
                        tmp = a1_red.tile([P, 512], f32, tag="a1t")
                        nc.vector.scalar_tensor_tensor(
                            tmp[:], s_sb[:, n0:n0 + 512],
                            b1_sb[:, mo:mo + 1], psh[:],
                            op0=Alu.mult, op1=Alu.add)
                        nc.vector.tensor_scalar_max(ht[:, mo, :], tmp[:],
                                                    0.0)

                    # MMW2: z2q = e4m3(d * (h.T @ W2')), node-major fp8
                    zq = z2q_pool.tile([P, 4, COUT], f8, tag="z2q")
                    ps3 = mmz_ps.tile([P, 4 * COUT], f32, name="mmz")
                    for ns in range(4):
                        psl = ps3[:, ns * COUT:(ns + 1) * COUT]
                        for mo in range(2):
                            nc.tensor.matmul(
                                psl, ht[:, mo, ns * P:(ns + 1) * P],
                                w2_sb[:, mo],
                                start=(mo == 0), stop=(mo == 1))
                        nc.vector.tensor_scalar_mul(
                            zq[:, ns, :], psl,
                            dz2_sb[:, nt * 4 + ns:nt * 4 + ns + 1])
                    # scalar-queue write: keeps this compute-dependent DMA
                    # from head-blocking the A stream on the sync queue
                    if nt < 2:
                        nc.scalar.dma_start(z2bA[:, nt], zq[:])
                    elif nt < 4:
                        nc.scalar.dma_start(z2bB[:, nt - 2], zq[:])
                    else:
                        nc.scalar.dma_start(z2bC[:, 0], zq[:])

                    if nt == 1:
                        nc.gpsimd.collective_compute(
                            "AllGather", mybir.AluOpType.bypass,
                            ins=[z2bA[:]], outs=[z2gA[:]],
                            replica_groups=[list(range(NCORES))])
                        # preload AGG2's z2 tiles as soon as each gather
                        # lands (gpsimd queue orders them right behind it)
                        for g in range(NCORES):
                            zp = z2pre_pool.tile([P, 2, 4, COUT], f8,
                                                 tag="z2preA")
                            nc.gpsimd.dma_start(zp[:], z2gA[g])
                            zga[g] = zp
                    elif nt == 3:
                        nc.gpsimd.collective_compute(
                            "AllGather", mybir.AluOpType.bypass,
                            ins=[z2bB[:]], outs=[z2gB[:]],
                            replica_groups=[list(range(NCORES))])
                        for g in range(NCORES):
                            zp = z2pre_pool.tile([P, 2, 4, COUT], f8,
                                                 tag="z2preB")
                            nc.gpsimd.dma_start(zp[:], z2gB[g])
                            zgb[g] = zp
                    elif nt == NT - 1:
                        nc.gpsimd.collective_compute(
                            "AllGather", mybir.AluOpType.bypass,
                            ins=[z2bC[:]], outs=[z2gC[:]],
                            replica_groups=[list(range(NCORES))])
                        for g in range(NCORES):
                            zp = z2pre_pool.tile([P, 1, 4, COUT], f8,
                                                 tag="z2preC")
                            nc.gpsimd.dma_start(zp[:], z2gC[g])
                            zgc[g] = zp
            xq_cm.__exit__(None, None, None)

            # ---- Phase 4: outT = d*contract(z2q, C) + b2' (x) s ----------
            # Hand-rolled k-outer loop: one PSUM bank per n-tile, so the
            # first gathered z2 tile starts compute while later gathers
            # are still in flight.  Pinned A blocks come from SBUF.
            with ExitStack() as ctx:
                a2_red = ctx.enter_context(tc.tile_pool(name="a2_red",
                                                        bufs=2))
                a2_ps = ctx.enter_context(
                    tc.tile_pool(name="a2_ps", bufs=1, space="PSUM"))

                psums = [a2_ps.tile([P, 512], f32, name=f"a2ps{n}")
                         for n in range(NT)]
                for qq in range(QQ):
                    ats = []
                    for n in range(NT):
                        if (qq, n) in pins:
                            ats.append(pins[(qq, n)])
                            continue
                        at = a2_kxn.tile([P, 32, 512], f8, tag="a2A")
                        nc.sync.dma_start(at[:], Ab[qq, n])
                        ats.append(at)
                    for oct_ in range(8):
                        q = 8 * qq + oct_
                        t, g = divmod(q, NCORES)
                        if t < 2:
                            zt = zga[g][:, t]
                        elif t < 4:
                            zt = zgb[g][:, t - 2]
                        else:
                            zt = zgc[g][:, 0]
                        for jp in range(2):
                            for n in range(NT):
                                nc.tensor.matmul(
                                    psums[n][:],
                                    zt[:, 2 * jp:2 * jp + 2, :],
                                    ats[n][:, oct_ * 4 + 2 * jp:
                                           oct_ * 4 + 2 * jp + 2, :],
                                    start=(q == 0 and jp == 0),
                                    stop=(q == KT - 1 and jp == 1),
                                    perf_mode=DR)

                for n in range(NT):
                    n0 = n * 512
                    tmp = a2_red.tile([P, 512], f32, tag="a2t")
                    osb = a2_red.tile([P, 512], f16, tag="a2o")
                    nc.vector.tensor_mul(tmp[:], psums[n][:],
                                         d_sb[:, n0:n0 + 512])
                    nc.vector.scalar_tensor_tensor(
                        osb[:], s_sb[:, n0:n0 + 512],
                        b2_sb[:, 0:1], tmp[:],
                        op0=Alu.mult, op1=Alu.add)
                    nc.scalar.dma_start(outT[:, 0, n0:n0 + 512], osb[:])

    nc.compile()
    return nc


def _preprocess(x, edge_index, W1, b1, W2, b2):
    import ml_dtypes

    x = np.asarray(x, dtype=np.float32)
    edge_index = np.asarray(edge_index)
    W1 = np.asarray(W1, dtype=np.float32)
    b1 = np.asarray(b1, dtype=np.float32)
    W2 = np.asarray(W2, dtype=np.float32)
    b2 = np.asarray(b2, dtype=np.float32)

    row = edge_index[0].astype(np.int64)
    col = edge_index[1].astype(np.int64)

    deg = np.bincount(col, minlength=N_REAL).astype(np.float32) + 1.0
    dinv = 1.0 / np.sqrt(deg)

    idx = np.arange(N_REAL, dtype=np.int64)
    pad_id = (idx // RBLK) * BLK + idx % RBLK  # real -> padded node id

    # Dense count matrix, transposed: CT[src, dst] = A[dst, src] + I
    CT = np.zeros((NPAD, NPAD), dtype=np.uint8)
    np.add.at(CT, (pad_id[row], pad_id[col]), 1)
    CT[pad_id, pad_id] += 1
    assert CT.max() <= 16, "count exceeds exact fp8e4m3 integer range"

    # s[c] = sum_r A_hat[c, r]; dinv at padded positions -> 0
    s_real = dinv * (np.bincount(col, weights=dinv[row],
                                 minlength=N_REAL).astype(np.float32) + dinv)
    s_pad = np.zeros(NPAD, dtype=np.float32)
    s_pad[pad_id] = s_real
    dinv_pad = np.zeros(NPAD, dtype=np.float32)
    dinv_pad[pad_id] = dinv

    # xq = e4m3(S1 * dinv * x), grouped 4 k-tiles per DMA block:
    # [mt//4][p][(mt%4)*4 + j][c] = xq[mt*512 + j*128 + p, c]
    x_pad = np.zeros((NPAD, CIN), dtype=np.float32)
    x_pad[pad_id] = x
    xq_full = np.clip(S1 * dinv_pad[:, None] * x_pad, -240.0, 240.0)
    xqT_t = np.ascontiguousarray(
        xq_full.reshape(KT // 4, 4, 4, P, CIN).transpose(0, 3, 1, 2, 4)
        .reshape(KT // 4, P, 16, CIN)
    ).astype(ml_dtypes.float8_e4m3)

    W1_t = np.ascontiguousarray(
        (W1 / S1).astype(np.float16)
        .reshape(CIN // P, P, CHID).transpose(1, 0, 2))
    W2_t = np.ascontiguousarray(
        (W2 * S2).astype(np.float16)
        .reshape(CHID // P, P, COUT).transpose(1, 0, 2))
    b1_t = np.ascontiguousarray(b1.reshape(CHID // P, P).T)
    b2_t = np.ascontiguousarray((b2 * S2).reshape(COUT // P, P).T)

    in_maps = []
    for g in range(NCORES):
        C_g = CT[:, g * BLK:(g + 1) * BLK]
        # [kt][nt][p][s][n] = C_g[kt*512 + s*128 + p, nt*512 + n],
        # then permute the kt axis into the device's q-order
        # (q -> physical kt = (q % NCORES) * NT + q // NCORES) and pack
        # 8 q's per stream block: [qq][nt][p][32][512] (16 KB/partition).
        perm = [(q % NCORES) * NT + q // NCORES for q in range(KT)]
        A_t = np.ascontiguousarray(
            C_g.reshape(KT, 4, P, NT, 512).transpose(0, 3, 2, 1, 4)[perm]
            .reshape(QQ, 8, NT, P, 4, 512).transpose(0, 2, 3, 1, 4, 5)
            .reshape(QQ, NT, P, 32, 512)
        ).astype(ml_dtypes.float8_e4m3)
        s_loc = s_pad[g * BLK:(g + 1) * BLK]
        d_loc = dinv_pad[g * BLK:(g + 1) * BLK]
        s_b = np.ascontiguousarray(
            np.broadcast_to(s_loc, (P, BLK))).astype(np.float16)
        d_b = np.ascontiguousarray(
            np.broadcast_to(d_loc, (P, BLK))).astype(np.float16)
        dz2_t = np.ascontiguousarray(d_loc.reshape(BLK // P, P).T)
        in_maps.append(dict(xqT=xqT_t, W1=W1_t, W2=W2_t, Ab=A_t,
                            sbc=s_b, dbc=d_b, dz2=dz2_t,
                            b1c=b1_t, b2c=b2_t))
    return in_maps


def _run(inputs, trace=False):
    global _compiled
    if _compiled is None:
        _compiled = _build_nc()
    nc = _compiled
    from concourse.bass_utils import run_bass_kernel_spmd

    in_maps = _preprocess(**inputs)
    res = run_bass_kernel_spmd(nc, in_maps, list(range(NCORES)), trace=trace)
    out = np.empty((N_REAL, COUT), dtype=np.float32)
    for g in range(NCORES):
        out[g * RBLK:(g + 1) * RBLK] = \
            res.results[g]["outT"][:, 0, :RBLK].T.astype(np.float32) / S2
    return out, res


def kernel(**inputs) -> np.ndarray:
    out, _ = _run(inputs, trace=False)
    return out
